# revision 20
# baseline (speedup 1.0000x reference)
"""CrossViewTransformer Bass kernel for 8 trn2 NeuronCores.

Problem (per batch element b of 4):
    q = (Wq @ top_b + bq)      # [32, 4096]
    k = (Wk @ side_b + bk)     # [32, 4096]
    v = (Wv @ side_b + bv)     # [256, 4096]
    E = softmax_over_keys(q.T @ k)        # [4096q, 4096k]
    out_b = top_b + (E @ v.T).T           # [256, 4096]

Sharding: 8 cores = (batch b = core//2) x (query half h = core%2).
Each core handles 2048 queries against all 4096 keys of its batch
element; no collectives. Weights replicated.

Key structural choices (v2, rebuilt from the 184us baseline's trace):
  - Inputs ship as f16 from the host (halves input DMA); the score path
    (q/k projections, q.T@k) stays f16 like the baseline; the value path
    is f16 -> bf16 (ex must be bf16 for range: exp(s) up to ~e^40).
  - bk is dropped exactly (softmax is invariant to per-query shifts:
    q.(k+bk) = q.k + const(q)); bv is folded into the residual on the
    host (softmax rows sum to 1 so E_norm @ (v+bv) = E_norm@v + bv).
  - The output stays in [query, channel] orientation end to end: av psum
    tiles are [128q, C+rowsum], the residual input tops ships as
    topT+bv in [q, C], the DRAM output is [q, C] f16 and the host
    transposes/casts. This removes every on-device transpose (the
    baseline spent 39us of DMA-transpose on the Sync engine).
  - Projections write the packed attention layouts directly via
    column-group matmul packing (tile_position=(0,32i)): k lands
    partition-packed for the 4-way row-group qk matmul, q lands
    replicated across the 4 row groups. No separate pack phase.
  - The main loop is a software pipeline over 32 (chunk, key-group)
    stages. ScalarE runs one 2048-element exp per stage (the ~64us hard
    floor: 8.4M exps at 1 elem/cycle/lane @ 1.2GHz); per stage the PE
    runs the next qk immediately after the exp that frees the sc psum,
    and av matmul work is drained from a quarter-stage work queue sized
    to never delay the next qk, which lets the prologue's av backlog
    (kept pending while the projection psum pool is still open) drain
    through per-stage PE slack. The epilogue runs entirely on DVE
    (reciprocal + one scalar_tensor_tensor per 128-query block), fused
    qb-major into each chunk's last key group so the out DMA overlaps
    the next chunk. ScalarE stays ~95% busy in steady state and the PE
    never idles >1us (HAM stays warm; the baseline oscillated at every
    chunk boundary, 43us throttled).

    Measured: 183.6us (prior baseline) -> 114.7us, rel err 3.1e-3.
    Steady stage period 2.64us = 1.97 exp + ~0.65 chain gap
    (exp(S)->qk(S+1)->exp(S+1): sc is single-buffered because PSUM is
    exactly full: sc 4 banks + av accumulators 4 banks). Attempting
    sc double-buffering via QC=256 (sc [128,4,256] x2 + av 2/chunk)
    crashes NRT unrecoverably - apparent PSUM hazard - see
    kernel_v4.py for the attempt.
"""

import sys

import numpy as np

B, C, H, W = 4, 256, 64, 64
N = H * W      # 4096 keys per batch element
C8 = 32
NCORES = 8
NQ = N // 2    # 2048 queries per core
QC = 512       # query chunk
QB = 128       # query block (matmul M)
KB = 128       # key block
NKB = N // KB  # 32 key blocks
NG = NKB // 4  # 8 groups of 4 packed key blocks
NCHUNK = NQ // QC  # 4
NST = NCHUNK * NG  # 32 pipeline stages

_BUILT = None


def _build():
    for p in ("/opt/trn_rl_repo", "/root/.axon_site/_ro/trn_rl_repo"):
        if p not in sys.path:
            sys.path.append(p)
    import concourse.bass as bass
    import concourse.tile as tile
    from concourse import bacc, mybir

    fp32 = mybir.dt.float32
    f16 = mybir.dt.float16
    bf16 = mybir.dt.bfloat16
    EXP = mybir.ActivationFunctionType.Exp
    ADD = mybir.AluOpType.add
    MULT = mybir.AluOpType.mult

    nc = bacc.Bacc("TRN2", target_bir_lowering=False, debug=False,
                   num_devices=NCORES)

    top_d = nc.dram_tensor("top", [C, NQ], f16, kind="ExternalInput").ap()
    side_d = nc.dram_tensor("side", [C, N], f16, kind="ExternalInput").ap()
    # topTbv and out ship in SBUF-native [p, a, c] layout (p-major) so the
    # DMA moves 8 KB contiguous per partition; the host permutes.
    tb_d = nc.dram_tensor("topTbv", [128, NQ // QB, C], f16,
                          kind="ExternalInput").ap()
    wqT_d = nc.dram_tensor("wqT", [C, C8], f16, kind="ExternalInput").ap()
    wkT_d = nc.dram_tensor("wkT", [C, C8], f16, kind="ExternalInput").ap()
    wvT_d = nc.dram_tensor("wvT", [C, C], f16, kind="ExternalInput").ap()
    bqr_d = nc.dram_tensor("bqr", [128, 1], fp32, kind="ExternalInput").ap()
    out_d = nc.dram_tensor("out", [128, NQ // QB, C], f16,
                           kind="ExternalOutput").ap()

    # channel dim split into 2 partition blocks of 128
    top_r3 = top_d.rearrange("(t p) n -> p t n", p=128)
    side_r3 = side_d.rearrange("(t p) n -> p t n", p=128)
    wqT_r3 = wqT_d.rearrange("(t p) m -> p t m", p=128)
    wkT_r3 = wkT_d.rearrange("(t p) m -> p t m", p=128)
    wvT_r3 = wvT_d.rearrange("(t p) m -> p t m", p=128)
    tb_r3 = tb_d
    out_r3 = out_d

    with tile.TileContext(nc) as tc:
        with tc.tile_pool(name="persist", bufs=1) as pers, \
             tc.tile_pool(name="work", bufs=1) as work:

            # ---- persistent SBUF tiles ----
            side_sb = pers.tile([128, 2, N], f16, tag="side")
            top_sb = pers.tile([128, 2, NQ], f16, tag="top")
            tb_sb = pers.tile([128, NQ // QB, C], f16, tag="tb")
            out_sb = pers.tile([128, NQ // QB, C], f16, tag="out")
            q_rep = pers.tile([128, NQ], f16, tag="q_rep")
            k_pack = pers.tile([128, NG, KB], f16, tag="k_pack")
            vT_b = pers.tile([128, NKB, C + 2], bf16, tag="vT")
            wq_sb = pers.tile([128, 2, C8], f16, tag="wq")
            wk_sb = pers.tile([128, 2, C8], f16, tag="wk")
            wv_sb = pers.tile([128, 2, C], f16, tag="wv")
            bq_sb = pers.tile([128, 1], fp32, tag="bq")
            dum_i = pers.tile([128, 1], fp32, tag="dum_i")
            dum_o = pers.tile([128, 1], fp32, tag="dum_o")

            # exp table preload: a dummy activation at t=0 pulls the
            # ~2.7us ACT_TABLE_LOAD into the DMA-wait window
            nc.gpsimd.memset(dum_i[:], 0.0)
            nc.scalar.activation(dum_o[:], dum_i[:], EXP)

            # vT's rowsum ones-column (col C; col C+1 stays 0 padding)
            nc.gpsimd.memset(vT_b[:, :, C:C + 2], 0.0)
            nc.gpsimd.memset(vT_b[:, :, C:C + 1], 1.0)

            # ---- input DMAs, strictly in order of first use; remainders
            # move as single transfers with multi-KB contiguous lines ----
            nc.sync.dma_start(side_sb[:, :, 0:QC], side_r3[:, :, 0:QC])
            nc.sync.dma_start(wk_sb[:], wkT_r3[:])
            nc.sync.dma_start(top_sb[:, :, 0:QC], top_r3[:, :, 0:QC])
            nc.sync.dma_start(wq_sb[:], wqT_r3[:])
            nc.sync.dma_start(wv_sb[:], wvT_r3[:])
            nc.sync.dma_start(bq_sb[:], bqr_d[:])
            nc.sync.dma_start(side_sb[:, :, QC:6 * QC],
                              side_r3[:, :, QC:6 * QC])
            nc.sync.dma_start(top_sb[:, :, QC:NQ], top_r3[:, :, QC:NQ])
            nc.sync.dma_start(side_sb[:, :, 6 * QC:N], side_r3[:, :, 6 * QC:N])
            nc.sync.dma_start(tb_sb[:], tb_r3[:])

            # ---- attention stage helpers ----
            scs = {}
            exs = {}
            avs = {}

            def emit_qk(S):
                qc, g = divmod(S, NG)
                sc = scs[S] = tc_psS.tile([128, 4, QC], fp32, tag="sc",
                                          bufs=1, name="sc")
                qsl = bass.ts(qc, QC)
                for i in range(4):
                    nc.tensor.matmul(sc[:, i, :],
                                     k_pack[32 * i:32 * (i + 1), g, :],
                                     q_rep[32 * i:32 * (i + 1), qsl],
                                     start=True, stop=True,
                                     tile_position=(32 * i, 0))

            def emit_exp(S):
                ex = exs[S] = work.tile([128, 4, QC], bf16, tag="ex",
                                        bufs=8, name="ex")
                nc.scalar.activation(ex[:], scs.pop(S)[:], EXP)

            def emit_epilogue_qb(qc, qb, av):
                a = 4 * qc + qb
                rc = work.tile([128, 1], fp32, tag="rc", bufs=8,
                               name=f"rc{qb}")
                nc.vector.reciprocal(rc[:], av[qb][:, C:C + 1])
                nc.vector.scalar_tensor_tensor(
                    out_sb[:, a, :], av[qb][:, 0:C], rc[:],
                    tb_sb[:, a, :], op0=MULT, op1=ADD)

            # av work is emitted in quarter-stage units (4 matmuls,
            # ~0.44us) pulled from a queue between qk and exp of later
            # stages, so the prologue's av backlog drains through the
            # PE's per-stage slack without ever delaying the next qk
            def emit_av_quarter(S, u):
                qc, g = divmod(S, NG)
                if g == 0 and u == 0:
                    avs[qc] = [tc_psA.tile([128, C + 2], fp32, tag="av",
                                           bufs=4, name=f"av{qb}")
                               for qb in range(4)]
                ex = exs[S]
                if u == 3:
                    exs.pop(S)
                if g < NG - 1:
                    j = 4 * g + u
                    for qb in range(4):
                        nc.tensor.matmul(avs[qc][qb][:],
                                         ex[:, u, bass.ts(qb, QB)],
                                         vT_b[:, j, :],
                                         start=(j == 0), stop=False)
                    return
                # final group of the chunk: qb-major so each query block's
                # accumulation finishes with its epilogue fused in
                qb = u
                av = avs[qc]
                for i in range(4):
                    nc.tensor.matmul(av[qb][:],
                                     ex[:, i, bass.ts(qb, QB)],
                                     vT_b[:, 4 * g + i, :],
                                     start=False, stop=(i == 3))
                emit_epilogue_qb(qc, qb, av)
                if qb == 3:
                    avs.pop(qc)
                    asl = bass.ts(qc, 4)
                    nc.sync.dma_start(out_r3[:, asl, :], out_sb[:, asl, :])

            with tc.tile_pool(name="ps_sc", bufs=1, space="PSUM") as tc_psS:
                # ---- prologue: projections straight into packed layouts
                with tc.tile_pool(name="ps_pro", bufs=1, space="PSUM") as psP:
                    # the two 128-channel halves (t) accumulate in PSUM;
                    # the 4 col-groups write disjoint partition ranges of
                    # the same bank (per-partition has_written state)
                    def emit_kproj(g):
                        kp = psP.tile([128, QC], fp32, tag="pp", bufs=4,
                                      name=f"kp{g}")
                        for i in range(4):
                            ksl = bass.ts(4 * g + i, KB)
                            for t in range(2):
                                nc.tensor.matmul(
                                    kp[32 * i:32 * (i + 1), 0:KB],
                                    wk_sb[:, t, :], side_sb[:, t, ksl],
                                    start=(t == 0), stop=(t == 1),
                                    tile_position=(0, 32 * i))
                        nc.vector.tensor_copy(k_pack[:, g, :], kp[:, 0:KB])

                    def emit_qproj(s):
                        pq = psP.tile([128, QC], fp32, tag="pp", bufs=4,
                                      name=f"pq{s}")
                        qsl = bass.ts(s, QC)
                        for i in range(4):
                            for t in range(2):
                                nc.tensor.matmul(
                                    pq[32 * i:32 * (i + 1), :],
                                    wq_sb[:, t, :], top_sb[:, t, qsl],
                                    start=(t == 0), stop=(t == 1),
                                    tile_position=(0, 32 * i))
                        nc.vector.tensor_scalar_add(q_rep[:, qsl], pq[:],
                                                    bq_sb[:])

                    def emit_vproj(j):
                        pv = psP.tile([128, QC], fp32, tag="pp", bufs=4,
                                      name=f"pv{j}")
                        jsl = bass.ts(j, KB)
                        for t in range(2):
                            nc.tensor.matmul(pv[:, 0:C],
                                             side_sb[:, t, jsl],
                                             wv_sb[:, t, :],
                                             start=(t == 0), stop=(t == 1))
                        nc.vector.tensor_copy(vT_b[:, j, 0:C], pv[:, 0:C])

                    # projections interleave with the first six qk/exp
                    # stages; pq(1..3) go last (first needed at stage 8)
                    emit_kproj(0)
                    emit_qproj(0)
                    emit_qk(0)
                    emit_exp(0)
                    emit_kproj(1)
                    emit_qk(1)
                    emit_exp(1)
                    emit_kproj(2)
                    emit_kproj(3)
                    for j in range(0, 4):
                        emit_vproj(j)
                    emit_qk(2)
                    emit_exp(2)
                    emit_kproj(4)
                    emit_kproj(5)
                    for j in range(4, 12):
                        emit_vproj(j)
                    emit_qk(3)
                    emit_exp(3)
                    emit_kproj(6)
                    emit_kproj(7)
                    for j in range(12, 20):
                        emit_vproj(j)
                    emit_qk(4)
                    emit_exp(4)
                    for j in range(20, 28):
                        emit_vproj(j)
                    emit_qk(5)
                    emit_exp(5)
                    for j in range(28, NKB):
                        emit_vproj(j)
                    for s in range(1, NCHUNK):
                        emit_qproj(s)

                # ---- main pipeline over the av quarter queue ----
                with tc.tile_pool(name="ps_av", bufs=1, space="PSUM") \
                        as tc_psA:
                    avq = [(S, u) for S in range(6) for u in range(4)]
                    for S in range(6, NST):
                        emit_qk(S)
                        n = 5 if len(avq) > 8 else 4
                        for _ in range(min(n, len(avq))):
                            emit_av_quarter(*avq.pop(0))
                        emit_exp(S)
                        avq.extend((S, u) for u in range(4))
                    for q in avq:
                        emit_av_quarter(*q)

    nc.compile()
    return nc


def _get_built():
    global _BUILT
    if _BUILT is None:
        _BUILT = _build()
    return _BUILT


def kernel(topview, sideview, Wq, bq, Wk, bk, Wv, bv):
    from concourse.bass_utils import run_bass_kernel_spmd

    top_f = np.asarray(topview, np.float32).reshape(B, C, N)
    side_f = np.asarray(sideview, np.float32).reshape(B, C, N)
    wqT = np.ascontiguousarray(np.asarray(Wq, np.float32).T
                               ).astype(np.float16)
    wkT = np.ascontiguousarray(np.asarray(Wk, np.float32).T
                               ).astype(np.float16)
    wvT = np.ascontiguousarray(np.asarray(Wv, np.float32).T
                               ).astype(np.float16)
    bqr = np.ascontiguousarray(
        np.tile(np.asarray(bq, np.float32), 4).reshape(128, 1))
    bv_f = np.asarray(bv, np.float32)
    # bk is dropped: softmax over keys is invariant to the per-query
    # shift q.bk. bv folds into the residual (softmax rows sum to 1).

    side16 = [np.ascontiguousarray(side_f[b]).astype(np.float16)
              for b in range(B)]

    in_maps = []
    for core in range(NCORES):
        b, h = core // 2, core % 2
        qsl = slice(h * NQ, (h + 1) * NQ)
        top_c = top_f[b, :, qsl]
        # topTbv in [p, a, c] device layout: q = a*128 + p
        tb = (top_c.T + bv_f[None, :]).reshape(NQ // QB, 128, C)
        in_maps.append({
            "top": np.ascontiguousarray(top_c).astype(np.float16),
            "side": side16[b],
            "topTbv": np.ascontiguousarray(
                tb.transpose(1, 0, 2)).astype(np.float16),
            "wqT": wqT, "wkT": wkT, "wvT": wvT, "bqr": bqr,
        })

    global _last_in_maps
    _last_in_maps = in_maps

    nc = _get_built()
    res = run_bass_kernel_spmd(nc, in_maps, core_ids=list(range(NCORES)))

    out = np.empty((B, C, N), dtype=np.float32)
    for core in range(NCORES):
        b, h = core // 2, core % 2
        # device out is [p, a, c]; q = a*128 + p -> [C, NQ]
        o = res.results[core]["out"].astype(np.float32)
        out[b, :, h * NQ:(h + 1) * NQ] = o.transpose(2, 1, 0).reshape(C, NQ)
    return out.reshape(B, C, H, W)
